# revision 1
# baseline (speedup 1.0000x reference)
"""Trainium2 Bass kernel for nn_NodeEmbDecoder (LSTM decoder + masked NN assignment).

Sharding: data-parallel over batch B=256 across 8 cores (32 rows each),
weights replicated and SBUF-resident. All activations kept transposed
([feature -> partitions, batch -> free]) so gate matmuls use weight tiles as
the stationary operand and the 32-row batch slice as the moving operand.

Everything is fp32 (true 4-pass PE matmuls) because the masked argmin over
node distances is decision-sensitive: lower precision flips assignments and
permutes whole output rows.
"""

import numpy as np

B, E, D, H, L, N = 256, 512, 128, 512, 2, 128
NCORES = 8
BL = B // NCORES  # 32 batch rows per core

_CACHE = {}


def _build(reps=1, debug=False):
    import concourse.bass as bass
    import concourse.bacc as bacc
    import concourse.tile as tile
    from concourse import mybir
    from concourse.bass import ds

    fp32 = mybir.dt.float32
    AF = mybir.ActivationFunctionType
    ALU = mybir.AluOpType

    nc = bacc.Bacc(None, target_bir_lowering=False, debug=True)

    # ---- DRAM I/O (per-core slices, host pre-laid-out) ----
    d_embT = nc.dram_tensor("embT", [128, 4, BL], fp32, kind="ExternalInput")
    d_nodeT = nc.dram_tensor("nodeT", [128, BL, N], fp32, kind="ExternalInput")
    d_W1 = nc.dram_tensor("W1T", [128, 4 * H], fp32, kind="ExternalInput")
    d_W2 = nc.dram_tensor("W2T", [128, 4 * 2 * H], fp32, kind="ExternalInput")
    d_Wih0 = nc.dram_tensor("Wih0T", [128, 1 * 4 * H], fp32, kind="ExternalInput")
    d_Whh0 = nc.dram_tensor("Whh0T", [128, 4 * 4 * H], fp32, kind="ExternalInput")
    d_Wih1 = nc.dram_tensor("Wih1T", [128, 4 * 4 * H], fp32, kind="ExternalInput")
    d_Whh1 = nc.dram_tensor("Whh1T", [128, 4 * 4 * H], fp32, kind="ExternalInput")
    d_Wo = nc.dram_tensor("WoT", [128, 4 * D], fp32, kind="ExternalInput")
    d_b1 = nc.dram_tensor("b1c", [128, 4], fp32, kind="ExternalInput")
    d_b2 = nc.dram_tensor("b2c", [128, 8], fp32, kind="ExternalInput")
    d_b0g = nc.dram_tensor("b0g", [128, 16], fp32, kind="ExternalInput")
    d_b1g = nc.dram_tensor("b1g", [128, 16], fp32, kind="ExternalInput")
    d_bo = nc.dram_tensor("boc", [128, 1], fp32, kind="ExternalInput")
    d_iota = nc.dram_tensor("iota128", [128, N], fp32, kind="ExternalInput")

    d_out = nc.dram_tensor("outT", [BL, N, D], fp32, kind="ExternalOutput")
    d_idx = nc.dram_tensor("idxs", [BL, N], fp32, kind="ExternalOutput")
    if debug:
        d_preds = nc.dram_tensor("predsD", [128, N * BL], fp32, kind="ExternalOutput")
        d_scores = nc.dram_tensor("scoresD", [BL, N * N], fp32, kind="ExternalOutput")

    with tile.TileContext(nc) as tc:
        import contextlib

        with contextlib.ExitStack() as ctx:
            wp = ctx.enter_context(tc.tile_pool(name="wp", bufs=1))
            st = ctx.enter_context(tc.tile_pool(name="st", bufs=1))
            ga = ctx.enter_context(tc.tile_pool(name="ga", bufs=2))
            ps = ctx.enter_context(tc.tile_pool(name="ps", bufs=6, space="PSUM"))
            ps2 = ctx.enter_context(tc.tile_pool(name="ps2", bufs=2, space="PSUM"))
            sc = ctx.enter_context(tc.tile_pool(name="sc", bufs=4))
            dr = ctx.enter_context(tc.tile_pool(name="dr", bufs=1, space="DRAM"))

            # ---- weights / constants into SBUF (once) ----
            W1 = wp.tile([128, 4, H], fp32)
            W2 = wp.tile([128, 4, 2 * H], fp32)
            Wih0 = wp.tile([128, 1, 4 * H], fp32)
            Whh0 = wp.tile([128, 4, 4 * H], fp32)
            Wih1 = wp.tile([128, 4, 4 * H], fp32)
            Whh1 = wp.tile([128, 4, 4 * H], fp32)
            Wo = wp.tile([128, 4, D], fp32)
            embT = wp.tile([128, 4, BL], fp32)
            nodeT = wp.tile([128, BL, N], fp32)
            b1c = wp.tile([128, 4], fp32)
            b2c = wp.tile([128, 8], fp32)
            b0g = wp.tile([128, 16], fp32)
            b1g = wp.tile([128, 16], fp32)
            boc = wp.tile([128, 1], fp32)
            iota = wp.tile([128, N], fp32)
            for dst, src in [
                (W1, d_W1), (W2, d_W2), (Wih0, d_Wih0), (Whh0, d_Whh0),
                (Wih1, d_Wih1), (Whh1, d_Whh1), (Wo, d_Wo), (embT, d_embT),
                (nodeT, d_nodeT), (b1c, d_b1), (b2c, d_b2), (b0g, d_b0g),
                (b1g, d_b1g), (boc, d_bo), (iota, d_iota),
            ]:
                nc.sync.dma_start(out=dst[:], in_=src[:])

            # persistent state
            h0S = st.tile([128, 4, BL], fp32)
            h1S = st.tile([128, 4, BL], fp32)
            c0S = st.tile([128, 4, BL], fp32)
            c1S = st.tile([128, 4, BL], fp32)
            xS = st.tile([128, BL], fp32)
            rT = st.tile([128, 4, BL], fp32)
            predsT = st.tile([128, N * BL], fp32)  # [d, t*BL + b]
            scoresQ = dr.tile([BL, N * N], fp32)  # DRAM scratch [b, t*N + n]
            msk = st.tile([BL, N], fp32)
            idxs = st.tile([BL, N], fp32)
            onesK = st.tile([128, 1], fp32)
            onesR = st.tile([1, N], fp32)
            ident = st.tile([128, 128], fp32)
            idxsT = st.tile([128, BL], fp32)

            nc.vector.memset(onesK[:], 1.0)
            nc.vector.memset(onesR[:], 1.0)
            from concourse.masks import make_identity
            make_identity(nc, ident[:])

            predsR = predsT[:].rearrange("p (t b) -> p t b", b=BL)

            import contextlib as _cl
            _loop = tc.For_i(0, reps, 1) if reps > 1 else _cl.nullcontext()
            with _loop:
                nc.vector.memset(c0S[:], 0.0)
                nc.vector.memset(c1S[:], 0.0)
                nc.vector.memset(xS[:], 0.0)
                nc.vector.memset(msk[:], 0.0)

                # ---- stage 1: FNN_in -> h0, h1 ----
                for m in range(4):
                    pt = ps.tile([128, BL], fp32, tag="ps")
                    for k in range(4):
                        nc.tensor.matmul(pt[:], W1[:, k, m * 128:(m + 1) * 128],
                                         embT[:, k, :], start=(k == 0), stop=(k == 3))
                    nc.scalar.activation(rT[:, m, :], pt[:], AF.Relu, bias=b1c[:, m:m + 1])
                for m in range(8):
                    pt = ps.tile([128, BL], fp32, tag="ps")
                    for k in range(4):
                        nc.tensor.matmul(pt[:], W2[:, k, m * 128:(m + 1) * 128],
                                         rT[:, k, :], start=(k == 0), stop=(k == 3))
                    dst = h0S[:, m, :] if m < 4 else h1S[:, m - 4, :]
                    nc.scalar.activation(dst, pt[:], AF.Identity, bias=b2c[:, m:m + 1])

                # ---- stage 2: LSTM decode, 128 steps ----
                def lstm_layer(WihS, WhhS, nih, x_aps, biasS, hS, cS):
                    # new h goes to temporaries first: hS must stay readable
                    # (old values) for all 16 m-tiles' Whh matmuls.
                    acts = {}
                    hnew = []
                    for m in range(16):
                        g, j = m // 4, m % 4
                        pt = ps.tile([128, BL], fp32, tag="ps")
                        for k in range(nih):
                            nc.tensor.matmul(pt[:], WihS[:, k, m * 128:(m + 1) * 128],
                                             x_aps[k], start=(k == 0), stop=False)
                        for k in range(4):
                            nc.tensor.matmul(pt[:], WhhS[:, k, m * 128:(m + 1) * 128],
                                             hS[:, k, :], start=False, stop=(k == 3))
                        a = ga.tile([128, BL], fp32, tag=f"a{g}{j}")
                        fn = AF.Tanh if g == 2 else AF.Sigmoid
                        nc.scalar.activation(a[:], pt[:], fn, bias=biasS[:, m:m + 1])
                        acts[(g, j)] = a
                        if g == 3:
                            i_, f_, g_, o_ = (acts[(0, j)], acts[(1, j)],
                                              acts[(2, j)], acts[(3, j)])
                            t1 = ga.tile([128, BL], fp32, tag=f"t1{j}")
                            nc.vector.tensor_tensor(t1[:], i_[:], g_[:], op=ALU.mult)
                            t2 = ga.tile([128, BL], fp32, tag=f"t2{j}")
                            nc.vector.tensor_tensor(t2[:], f_[:], cS[:, j, :], op=ALU.mult)
                            nc.vector.tensor_tensor(cS[:, j, :], t1[:], t2[:], op=ALU.add)
                            tct = ga.tile([128, BL], fp32, tag=f"tc{j}")
                            nc.scalar.activation(tct[:], cS[:, j, :], AF.Tanh)
                            hn = ga.tile([128, BL], fp32, tag=f"hn{j}")
                            nc.vector.tensor_tensor(hn[:], o_[:], tct[:], op=ALU.mult)
                            hnew.append(hn)
                    for j in range(4):
                        nc.vector.tensor_copy(hS[:, j, :], hnew[j][:])

                with tc.For_i(0, N, 1) as t:
                    lstm_layer(Wih0, Whh0, 1, [xS[:]], b0g, h0S, c0S)
                    lstm_layer(Wih1, Whh1, 4, [h0S[:, k, :] for k in range(4)],
                               b1g, h1S, c1S)
                    pt = ps.tile([128, BL], fp32, tag="ps")
                    for k in range(4):
                        nc.tensor.matmul(pt[:], Wo[:, k, :], h1S[:, k, :],
                                         start=(k == 0), stop=(k == 3))
                    nc.scalar.activation(xS[:], pt[:], AF.Identity, bias=boc[:])
                    nc.vector.tensor_copy(predsT[:, ds(t * BL, BL)], xS[:])

                # ---- stage 3: scores[b][t,n] = <pred_t, e_n> - 0.5||e_n||^2 ----
                for b in range(BL):
                    sqb = sc.tile([128, N], fp32, tag="sqb")
                    nc.scalar.activation(sqb[:], nodeT[:, b, :], AF.Square)
                    npt = ps2.tile([1, N], fp32, tag="x2")
                    nc.tensor.matmul(npt[:], onesK[:], sqb[:], start=True, stop=True)
                    neghb = sc.tile([1, N], fp32, tag="neghb")
                    nc.scalar.activation(neghb[:], npt[:], AF.Copy, scale=-0.5)
                    dpt = ps2.tile([128, N], fp32, tag="x2")
                    nc.tensor.matmul(dpt[:], onesR[:], neghb[:],
                                     start=True, stop=False)
                    nc.tensor.matmul(dpt[:], predsR[:, :, b], nodeT[:, b, :],
                                     start=False, stop=True)
                    sb = sc.tile([128, N], fp32, tag="scb")
                    nc.vector.tensor_copy(sb[:], dpt[:])
                    nc.sync.dma_start(out=scoresQ[b:b + 1, :], in_=sb[:])

                # ---- stage 4: sequential masked argmax over n ----
                mx8 = st.tile([BL, 8], fp32)
                ix8 = st.tile([BL, 8], mybir.dt.uint32)
                ixf = st.tile([BL, 1], fp32)
                mtile = st.tile([BL, N], fp32)
                eqm = st.tile([BL, N], fp32)
                for t in range(N):
                    cur = sc.tile([BL, N], fp32, tag="cur")
                    nc.sync.dma_start(out=cur[:], in_=scoresQ[0:BL, t * N:(t + 1) * N])
                    nc.vector.tensor_tensor(mtile[:], cur[:], msk[:], op=ALU.add)
                    nc.vector.max(mx8[:], mtile[:])
                    nc.vector.max_index(ix8[:], mx8[:], mtile[:])
                    nc.vector.tensor_copy(ixf[:], ix8[:, 0:1])
                    nc.vector.tensor_copy(idxs[:, t:t + 1], ixf[:])
                    nc.vector.tensor_scalar(eqm[:], iota[0:BL, :], ixf[:], -1e30,
                                            op0=ALU.is_equal, op1=ALU.mult)
                    nc.vector.tensor_tensor(msk[:], msk[:], eqm[:], op=ALU.add)

                # ---- stage 5: permute preds into output slots ----
                ipt = ps2.tile([128, BL], fp32, tag="x2")
                nc.tensor.transpose(ipt[:], idxs[:], ident[0:BL, 0:BL])
                nc.vector.tensor_copy(idxsT[:], ipt[:])
                for b in range(BL):
                    tpt = ps2.tile([128, 128], fp32, tag="x2")
                    nc.tensor.transpose(tpt[:], predsR[:, :, b], ident[:])
                    pb = sc.tile([128, 128], fp32, tag="pb")
                    nc.vector.tensor_copy(pb[:], tpt[:])
                    oh = sc.tile([128, N], fp32, tag="oh")
                    nc.vector.tensor_scalar(oh[:], iota[:], idxsT[:, b:b + 1], None,
                                            op0=ALU.is_equal)
                    opt = ps2.tile([N, D], fp32, tag="x2")
                    nc.tensor.matmul(opt[:], oh[:], pb[:], start=True, stop=True)
                    ob = sc.tile([N, D], fp32, tag="ob")
                    nc.vector.tensor_copy(ob[:], opt[:])
                    nc.sync.dma_start(out=d_out[b], in_=ob[:])

            nc.sync.dma_start(out=d_idx[:], in_=idxs[:])
            if debug:
                nc.sync.dma_start(out=d_preds[:], in_=predsT[:])
                nc.sync.dma_start(out=d_scores[:], in_=scoresQ[:])

    nc.finalize()
    return nc


def _prep_w(W):
    # torch-Linear weight [M_out, K_in] -> stationary lhsT sbuf layout
    # [128, K/128, M]:  sb[p, k, m] = W[m, k*128+p]
    M, K = W.shape
    kk = K // 128
    return np.ascontiguousarray(
        W.T.reshape(kk, 128, M).transpose(1, 0, 2).reshape(128, kk * M)
    ).astype(np.float32)


def _prep_bias_cols(b):
    # [M] -> [128, M/128] with col j = b[j*128:(j+1)*128]
    return np.ascontiguousarray(b.reshape(-1, 128).T).astype(np.float32)


def prepare_in_maps(emb, node_emb_encoded, W1, b1, W2, b2,
                    Wih0, Whh0, bih0, bhh0, Wih1, Whh1, bih1, bhh1,
                    Wo, bo):
    shared = {
        "W1T": _prep_w(np.asarray(W1)).reshape(128, 4, H),
        "W2T": _prep_w(np.asarray(W2)).reshape(128, 4, 2 * H).reshape(128, 8 * H),
        "Wih0T": _prep_w(np.asarray(Wih0)),
        "Whh0T": _prep_w(np.asarray(Whh0)),
        "Wih1T": _prep_w(np.asarray(Wih1)),
        "Whh1T": _prep_w(np.asarray(Whh1)),
        "WoT": _prep_w(np.asarray(Wo)),
        "b1c": _prep_bias_cols(np.asarray(b1)),
        "b2c": _prep_bias_cols(np.asarray(b2)),
        "b0g": _prep_bias_cols(np.asarray(bih0) + np.asarray(bhh0)),
        "b1g": _prep_bias_cols(np.asarray(bih1) + np.asarray(bhh1)),
        "boc": np.asarray(bo).reshape(128, 1).astype(np.float32),
        "iota128": np.tile(np.arange(N, dtype=np.float32), (128, 1)),
    }
    in_maps = []
    for c in range(NCORES):
        sl = slice(c * BL, (c + 1) * BL)
        emb_sl = np.asarray(emb[sl], dtype=np.float32)
        node_sl = np.asarray(node_emb_encoded[sl], dtype=np.float32)
        m = dict(shared)
        m["embT"] = np.ascontiguousarray(
            emb_sl.T.reshape(4, 128, BL).transpose(1, 0, 2))
        m["nodeT"] = np.ascontiguousarray(node_sl.transpose(2, 0, 1))
        in_maps.append(m)
    return in_maps


def run(inputs, reps=1, debug=False):
    from concourse.bass_utils import run_bass_kernel_spmd
    key = (reps, debug)
    if key not in _CACHE:
        _CACHE[key] = _build(reps=reps, debug=debug)
    nc = _CACHE[key]
    in_maps = prepare_in_maps(
        inputs["emb"], inputs["node_emb_encoded"], inputs["W1"], inputs["b1"],
        inputs["W2"], inputs["b2"], inputs["Wih0"], inputs["Whh0"],
        inputs["bih0"], inputs["bhh0"], inputs["Wih1"], inputs["Whh1"],
        inputs["bih1"], inputs["bhh1"], inputs["Wo"], inputs["bo"])
    res = run_bass_kernel_spmd(nc, in_maps, list(range(NCORES)))
    return res.results


def kernel(**inputs) -> np.ndarray:
    results = run(inputs, reps=1, debug=False)
    out = np.concatenate([r["outT"] for r in results], axis=0)
    return out.astype(np.float32)



# revision 19
# speedup vs baseline: 15.7915x; 15.7915x over previous
"""Trainium2 Bass kernel for nn_NodeEmbDecoder (LSTM decoder + masked NN assignment).

Sharding: data-parallel over batch B=256 across 8 cores (32 rows each),
weights replicated and SBUF-resident. Activations transposed
([feature -> partitions, batch -> free]); weight tiles are the stationary
matmul operand, the 32-row batch slice is the moving operand.

The recurrent matmuls run as bf16 hi/lo split pairs (W = W_hi + W_lo,
x = x_hi + x_lo; products W_hi*x_hi + W_hi*x_lo + W_lo*x_hi accumulated in
fp32 PSUM, ~2^-16 relative error). Plain fp32 matmuls at free-dim 32 run
~12x slower on the PE (cold-clock LOW_HIGH pairs), and the ~1e-5 pred error
of the split form preserves every masked-argmin decision (verified: 0 index
flips vs the fp32 reference; plain bf16 flips 6-8 and fails). Everything
around the decisions (FNN init, distance scores, argmin, permutation) stays
fp32.
"""

import numpy as np

B, E, D, H, L, N = 256, 512, 128, 512, 2, 128
NCORES = 8
BL = B // NCORES  # 32 batch rows per core
UNROLL = 4        # decode steps per hardware-loop iteration

_CACHE = {}

# ---- dram buffer layouts (columns) ----
# bf16 weight buffer: per weight, hi plane then lo plane, each [128, kk*M]
_WB_SEGS = [("Wih0", 1, 4 * H), ("Whh0", 4, 4 * H),
            ("Wih1", 4, 4 * H), ("Whh1", 4, 4 * H), ("Wo", 4, D)]
_WB_COLS = sum(2 * kk * m for _, kk, m in _WB_SEGS)
# fp32 misc buffer
_WF_SEGS = [("W1", 4 * H), ("W2", 4 * 2 * H), ("b1c", 4), ("b2c", 8),
            ("bb0", 4 * H // 4), ("bb1", 4 * H // 4), ("boc", 1),
            ("iota", N)]
_WF_COLS = sum(c for _, c in _WF_SEGS)
_DAT_COLS = 4 * BL + BL * N + N  # embT + nodeT + mask-init rows (0:BL)


def _build(reps=1, debug=False):
    import concourse.bass as bass
    import concourse.bacc as bacc
    import concourse.tile as tile
    from concourse import mybir
    from concourse.bass import ds

    fp32 = mybir.dt.float32
    bf16 = mybir.dt.bfloat16
    AF = mybir.ActivationFunctionType
    ALU = mybir.AluOpType

    nc = bacc.Bacc(None, target_bir_lowering=False, debug=False)

    d_wb = nc.dram_tensor("wb", [128, _WB_COLS], bf16, kind="ExternalInput")
    d_wf = nc.dram_tensor("wf", [128, _WF_COLS], fp32, kind="ExternalInput")
    d_dat = nc.dram_tensor("dat", [128, _DAT_COLS], fp32, kind="ExternalInput")
    d_out = nc.dram_tensor("outT", [BL, N, D], fp32, kind="ExternalOutput")
    if debug:
        d_idx = nc.dram_tensor("idxs", [BL, N], fp32, kind="ExternalOutput")
        d_preds = nc.dram_tensor("predsD", [128, N * BL], fp32,
                                 kind="ExternalOutput")
        d_dbg = nc.dram_tensor("dbg", [128, 6 * 128], fp32,
                               kind="ExternalOutput")

    def wb_off(name):
        o = 0
        for n_, kk, m in _WB_SEGS:
            if n_ == name:
                return o, kk, m
            o += 2 * kk * m
        raise KeyError(name)

    def wf_off(name):
        o = 0
        for n_, c in _WF_SEGS:
            if n_ == name:
                return o, c
            o += c
        raise KeyError(name)

    with tile.TileContext(nc) as tc:
        import contextlib

        with contextlib.ExitStack() as ctx:
            wp = ctx.enter_context(tc.tile_pool(name="wp", bufs=1))
            st = ctx.enter_context(tc.tile_pool(name="st", bufs=1))
            ga = ctx.enter_context(tc.tile_pool(name="ga", bufs=2))
            psA = ctx.enter_context(tc.tile_pool(name="psA", bufs=2, space="PSUM"))
            psB = ctx.enter_context(tc.tile_pool(name="psB", bufs=2, space="PSUM"))
            psW = ctx.enter_context(tc.tile_pool(name="psW", bufs=2, space="PSUM"))
            ps2 = ctx.enter_context(tc.tile_pool(name="ps2", bufs=2, space="PSUM"))
            sc = ctx.enter_context(tc.tile_pool(name="sc", bufs=4))
            dr = ctx.enter_context(tc.tile_pool(name="dr", bufs=1, space="DRAM"))

            # ---- weights / constants into SBUF (once) ----
            def wtile(name):
                o, kk, m = wb_off(name)
                hi = wp.tile([128, kk, m], bf16, name=name + "h")
                lo = wp.tile([128, kk, m], bf16, name=name + "l")
                nc.sync.dma_start(
                    out=hi[:], in_=d_wb[:, o:o + kk * m].rearrange(
                        "p (k m) -> p k m", k=kk))
                nc.sync.dma_start(
                    out=lo[:], in_=d_wb[:, o + kk * m:o + 2 * kk * m].rearrange(
                        "p (k m) -> p k m", k=kk))
                return hi, lo

            Wih0h, Wih0l = wtile("Wih0")
            Whh0h, Whh0l = wtile("Whh0")
            Wih1h, Wih1l = wtile("Wih1")
            Whh1h, Whh1l = wtile("Whh1")
            Woh, Wol = wtile("Wo")

            def ftile(name, shape=None):
                o, c = wf_off(name)
                t = wp.tile([128, c] if shape is None else shape, fp32, name=name)
                src = d_wf[:, o:o + c]
                if shape is not None and len(shape) == 3:
                    src = src.rearrange("p (k m) -> p k m", k=shape[1])
                nc.sync.dma_start(out=t[:], in_=src)
                return t

            W1S = ftile("W1", [128, 4, H])
            W2S = ftile("W2", [128, 4, 2 * H])
            b1c = ftile("b1c")
            b2c = ftile("b2c")
            bb0 = ftile("bb0")   # [128, 512] gate bias, (g,j,b)-broadcast
            bb1 = ftile("bb1")
            boc = ftile("boc")
            iota = ftile("iota")  # [128, N], each row 0..N-1

            embT = wp.tile([128, 4, BL], fp32)
            nodeT = wp.tile([128, BL, N], fp32)
            mskI = wp.tile([BL, N], fp32)
            nc.sync.dma_start(out=embT[:], in_=d_dat[:, 0:4 * BL].rearrange(
                "p (k b) -> p k b", k=4))
            nc.sync.dma_start(
                out=nodeT[:], in_=d_dat[:, 4 * BL:4 * BL + BL * N].rearrange(
                    "p (b n) -> p b n", b=BL))
            nc.sync.dma_start(out=mskI[:], in_=d_dat[0:BL, 4 * BL + BL * N:])

            # persistent state
            h0f = st.tile([128, 128], fp32)   # [p, (j,b)]
            h1f = st.tile([128, 128], fp32)
            c0S = st.tile([128, 128], fp32)
            c1S = st.tile([128, 128], fp32)
            h0h = st.tile([128, 128], bf16)
            h0l = st.tile([128, 128], bf16)
            h1h = st.tile([128, 128], bf16)
            h1l = st.tile([128, 128], bf16)
            xh = st.tile([128, BL], bf16)
            xl = st.tile([128, BL], bf16)
            predsT = st.tile([128, N * BL], fp32)  # [d, t*BL + b]
            scoresQ = dr.tile([BL, N * N], fp32)   # DRAM scratch [b, t*N + n]
            msk = st.tile([BL, N], fp32)
            idxsF = st.tile([BL, N], fp32)
            idxsT = st.tile([128, BL], fp32)
            ident = st.tile([128, 128], fp32)

            from concourse.masks import make_identity
            make_identity(nc, ident[:])

            predsR = predsT[:].rearrange("p (t b) -> p t b", b=BL)

            import contextlib as _cl
            _loop = tc.For_i(0, reps, 1) if reps > 1 else _cl.nullcontext()
            with _loop:
                nc.vector.memset(c0S[:], 0.0)
                nc.vector.memset(c1S[:], 0.0)
                nc.vector.memset(xh[:], 0.0)
                nc.vector.memset(xl[:], 0.0)

                # ---- stage 1: FNN_in -> h0f, h1f (fp32) ----
                p1 = psA.tile([128, 512], fp32, tag="G0")
                nc.vector.memset(p1[:], 0.0)
                for m in range(4):
                    for k in range(4):
                        nc.tensor.matmul(p1[:, m * BL:(m + 1) * BL],
                                         W1S[:, k, m * 128:(m + 1) * 128],
                                         embT[:, k, :], start=False,
                                         stop=(m == 3 and k == 3),
                                         skip_group_check=True)
                rT = st.tile([128, 4, BL], fp32)
                for m in range(4):
                    nc.scalar.activation(rT[:, m, :], p1[:, m * BL:(m + 1) * BL],
                                         AF.Relu, bias=b1c[:, m:m + 1])
                p2 = psA.tile([128, 512], fp32, tag="G0")
                nc.vector.memset(p2[:], 0.0)
                for m in range(8):
                    for k in range(4):
                        nc.tensor.matmul(p2[:, m * BL:(m + 1) * BL],
                                         W2S[:, k, m * 128:(m + 1) * 128],
                                         rT[:, k, :], start=False,
                                         stop=(m == 7 and k == 3),
                                         skip_group_check=True)
                for m in range(8):
                    dst = h0f if m < 4 else h1f
                    j = m % 4
                    nc.scalar.activation(dst[:, j * BL:(j + 1) * BL],
                                         p2[:, m * BL:(m + 1) * BL],
                                         AF.Identity, bias=b2c[:, m:m + 1])
                for hf, hh_, hl_ in ((h0f, h0h, h0l), (h1f, h1h, h1l)):
                    nc.vector.tensor_copy(hh_[:], hf[:])
                    nc.vector.tensor_tensor(hl_[:], hf[:], hh_[:], op=ALU.subtract)
                if debug:
                    nc.sync.dma_start(out=d_dbg[:, 0:128], in_=h0f[:])
                    nc.sync.dma_start(out=d_dbg[:, 128:256], in_=h1f[:])

                # ---- stage 2: LSTM decode, N steps ----
                def gate_mms(G, Wh, Wl, hh_, hl_):
                    # hh terms (use old h); ih terms come later via gate_mms_ih
                    for g in range(4):
                        for j in range(4):
                            dst = G[:, (g * 4 + j) * BL:(g * 4 + j + 1) * BL]
                            mm = slice(g * H + j * 128, g * H + (j + 1) * 128)
                            for k in range(4):
                                nc.tensor.matmul(dst, Wh[:, k, mm],
                                                 hh_[:, k * BL:(k + 1) * BL],
                                                 start=False, stop=False,
                                                 skip_group_check=True)
                                nc.tensor.matmul(dst, Wh[:, k, mm],
                                                 hl_[:, k * BL:(k + 1) * BL],
                                                 start=False, stop=False,
                                                 skip_group_check=True)
                                nc.tensor.matmul(dst, Wl[:, k, mm],
                                                 hh_[:, k * BL:(k + 1) * BL],
                                                 start=False, stop=False,
                                                 skip_group_check=True)

                def gate_mms_ih(G, Wh, Wl, xh_, xl_):
                    for g in range(4):
                        for j in range(4):
                            dst = G[:, (g * 4 + j) * BL:(g * 4 + j + 1) * BL]
                            mm = slice(g * H + j * 128, g * H + (j + 1) * 128)
                            for k in range(4):
                                xhk = xh_[:, k * BL:(k + 1) * BL]
                                xlk = xl_[:, k * BL:(k + 1) * BL]
                                last = (g == 3 and j == 3 and k == 3)
                                nc.tensor.matmul(dst, Wh[:, k, mm], xhk,
                                                 start=False, stop=False,
                                                 skip_group_check=True)
                                nc.tensor.matmul(dst, Wh[:, k, mm], xlk,
                                                 start=False, stop=False,
                                                 skip_group_check=True)
                                nc.tensor.matmul(dst, Wl[:, k, mm], xhk,
                                                 start=False, stop=last,
                                                 skip_group_check=True)

                def lstm_tail(G, bb, cS, hf, hh_, hl_, tag):
                    P = ga.tile([128, 512], fp32, tag=f"P{tag}")
                    nc.vector.tensor_tensor(P[:], G[:], bb[:], op=ALU.add)
                    A = ga.tile([128, 512], fp32, tag=f"A{tag}")
                    nc.scalar.activation(A[:, 0:256], P[:, 0:256], AF.Sigmoid)
                    nc.scalar.activation(A[:, 256:384], P[:, 256:384], AF.Tanh)
                    nc.scalar.activation(A[:, 384:512], P[:, 384:512], AF.Sigmoid)
                    t1 = ga.tile([128, 128], fp32, tag=f"t1{tag}")
                    nc.vector.tensor_tensor(t1[:], A[:, 0:128], A[:, 256:384],
                                            op=ALU.mult)
                    t2 = ga.tile([128, 128], fp32, tag=f"t2{tag}")
                    nc.vector.tensor_tensor(t2[:], A[:, 128:256], cS[:],
                                            op=ALU.mult)
                    nc.vector.tensor_tensor(cS[:], t1[:], t2[:], op=ALU.add)
                    tct = ga.tile([128, 128], fp32, tag=f"tc{tag}")
                    nc.scalar.activation(tct[:], cS[:], AF.Tanh)
                    nc.vector.tensor_tensor(hf[:], A[:, 384:512], tct[:],
                                            op=ALU.mult)
                    nc.vector.tensor_copy(hh_[:], hf[:])
                    nc.vector.tensor_tensor(hl_[:], hf[:], hh_[:],
                                            op=ALU.subtract)

                def step(t):
                    G0 = psA.tile([128, 512], fp32, tag="G0")
                    nc.vector.memset(G0[:], 0.0)
                    # L0: ih first (x ready from prev step), then hh
                    for g in range(4):
                        for j in range(4):
                            dst = G0[:, (g * 4 + j) * BL:(g * 4 + j + 1) * BL]
                            mm = slice(g * H + j * 128, g * H + (j + 1) * 128)
                            nc.tensor.matmul(dst, Wih0h[:, 0, mm], xh[:],
                                             start=False, stop=False,
                                             skip_group_check=True)
                            nc.tensor.matmul(dst, Wih0h[:, 0, mm], xl[:],
                                             start=False, stop=False,
                                             skip_group_check=True)
                            nc.tensor.matmul(dst, Wih0l[:, 0, mm], xh[:],
                                             start=False, stop=False,
                                             skip_group_check=True)
                            for k in range(4):
                                hhk = h0h[:, k * BL:(k + 1) * BL]
                                hlk = h0l[:, k * BL:(k + 1) * BL]
                                last = (g == 3 and j == 3 and k == 3)
                                nc.tensor.matmul(dst, Whh0h[:, k, mm], hhk,
                                                 start=False, stop=False,
                                                 skip_group_check=True)
                                nc.tensor.matmul(dst, Whh0h[:, k, mm], hlk,
                                                 start=False, stop=False,
                                                 skip_group_check=True)
                                nc.tensor.matmul(dst, Whh0l[:, k, mm], hhk,
                                                 start=False, stop=last,
                                                 skip_group_check=True)
                    # L1 hh (uses h1 from prev step) — keeps PE busy while
                    # the L0 tail produces h0
                    G1 = psB.tile([128, 512], fp32, tag="G1")
                    nc.vector.memset(G1[:], 0.0)
                    gate_mms(G1, Whh1h, Whh1l, h1h[:], h1l[:])
                    lstm_tail(G0, bb0, c0S, h0f, h0h, h0l, "0")
                    # L1 ih (needs new h0)
                    gate_mms_ih(G1, Wih1h, Wih1l, h0h[:], h0l[:])
                    lstm_tail(G1, bb1, c1S, h1f, h1h, h1l, "1")
                    # Wo -> pred (bf16 split), bias via DVE, write predsT
                    pw = psW.tile([128, BL], fp32, tag="W")
                    for k in range(4):
                        hhk = h1h[:, k * BL:(k + 1) * BL]
                        hlk = h1l[:, k * BL:(k + 1) * BL]
                        nc.tensor.matmul(pw[:], Woh[:, k, :], hhk,
                                         start=(k == 0), stop=False)
                        nc.tensor.matmul(pw[:], Woh[:, k, :], hlk,
                                         start=False, stop=False)
                        nc.tensor.matmul(pw[:], Wol[:, k, :], hhk,
                                         start=False, stop=(k == 3))
                    xs = predsT[:, ds(t * BL, BL)]
                    nc.vector.tensor_scalar(xs, pw[:], boc[:], None, op0=ALU.add)
                    nc.vector.tensor_copy(xh[:], xs)
                    nc.vector.tensor_tensor(xl[:], xs, xh[:], op=ALU.subtract)

                if debug:
                    step(0)
                    nc.sync.dma_start(out=d_dbg[:, 256:384], in_=h0f[:])
                    nc.sync.dma_start(out=d_dbg[:, 384:512], in_=h1f[:])
                    nc.sync.dma_start(out=d_dbg[:, 512:640], in_=c0S[:])
                    nc.sync.dma_start(out=d_dbg[:, 640:768], in_=c1S[:])
                    with tc.For_i(0, (N - 1) // UNROLL, 1,
                                  hint_engines=(mybir.EngineType.PE,)) as i:
                        for u in range(UNROLL):
                            step(1 + i * UNROLL + u)
                    nfull = 1 + (N - 1) // UNROLL * UNROLL
                    for u in range(N - nfull):
                        step(nfull + u)
                else:
                    with tc.For_i(0, N // UNROLL, 1,
                                  hint_engines=(mybir.EngineType.PE,)) as i:
                        for u in range(UNROLL):
                            step(i * UNROLL + u)

                # ---- stage 3: scores[b][t,n] = <p_t, e_n>; msk init from host ----
                nc.vector.tensor_copy(msk[:], mskI[:])
                for b in range(BL):
                    dpt = ps2.tile([128, N], fp32, tag="x2")
                    nc.tensor.matmul(dpt[:], predsR[:, :, b], nodeT[:, b, :],
                                     start=True, stop=True)
                    sb = sc.tile([128, N], fp32, tag="scb")
                    nc.vector.tensor_copy(sb[:], dpt[:])
                    nc.sync.dma_start(out=scoresQ[b:b + 1, :], in_=sb[:])

                # ---- stage 4: sequential masked argmax over n ----
                mx8 = st.tile([BL, 8], fp32)
                ix8 = st.tile([BL, 8], mybir.dt.uint32)
                mtile = st.tile([BL, N], fp32)
                eqm = st.tile([BL, N], fp32)
                for t in range(N):
                    cur = sc.tile([BL, N], fp32, tag="cur")
                    nc.sync.dma_start(out=cur[:],
                                      in_=scoresQ[0:BL, t * N:(t + 1) * N])
                    nc.vector.tensor_tensor(mtile[:], cur[:], msk[:], op=ALU.add)
                    nc.vector.max(mx8[:], mtile[:])
                    nc.vector.max_index(ix8[:], mx8[:], mtile[:])
                    nc.vector.tensor_copy(idxsF[:, t:t + 1], ix8[:, 0:1])
                    nc.vector.tensor_scalar(eqm[:], iota[0:BL, :],
                                            idxsF[:, t:t + 1], -1e30,
                                            op0=ALU.is_equal, op1=ALU.mult)
                    nc.vector.tensor_tensor(msk[:], msk[:], eqm[:], op=ALU.add)

                # ---- stage 5: permute preds into output slots ----
                ipt = ps2.tile([128, BL], fp32, tag="x2")
                nc.tensor.transpose(ipt[:], idxsF[:], ident[0:BL, 0:BL])
                nc.vector.tensor_copy(idxsT[:], ipt[:])
                for b in range(BL):
                    tpt = ps2.tile([128, 128], fp32, tag="x2")
                    nc.tensor.transpose(tpt[:], predsR[:, :, b], ident[:])
                    pb = sc.tile([128, 128], fp32, tag="pb")
                    nc.vector.tensor_copy(pb[:], tpt[:])
                    oh = sc.tile([128, N], fp32, tag="oh")
                    nc.vector.tensor_scalar(oh[:], iota[:], idxsT[:, b:b + 1],
                                            None, op0=ALU.is_equal)
                    opt = ps2.tile([N, D], fp32, tag="x2")
                    nc.tensor.matmul(opt[:], oh[:], pb[:], start=True, stop=True)
                    ob = sc.tile([N, D], fp32, tag="ob")
                    nc.vector.tensor_copy(ob[:], opt[:])
                    nc.sync.dma_start(out=d_out[b], in_=ob[:])

            if debug:
                nc.sync.dma_start(out=d_idx[:], in_=idxsF[:])
                nc.sync.dma_start(out=d_preds[:], in_=predsT[:])

    nc.finalize()
    return nc


def _bf16(x):
    import ml_dtypes
    return x.astype(ml_dtypes.bfloat16)


def _prep_w(W):
    # torch-Linear weight [M_out, K_in] -> stationary lhsT sbuf layout
    # [128, K/128, M]:  sb[p, k, m] = W[m, k*128+p]
    M, K = W.shape
    kk = K // 128
    return np.ascontiguousarray(
        W.T.reshape(kk, 128, M).transpose(1, 0, 2).reshape(128, kk * M)
    ).astype(np.float32)


def _prep_bias_cols(b):
    # [M] -> [128, M/128] with col j = b[j*128:(j+1)*128]
    return np.ascontiguousarray(b.reshape(-1, 128).T).astype(np.float32)


def _prep_bias_bc(b):
    # [2048] -> [128, 512]: value at [p, (g*4+j)*32 + i] = b[g*512 + j*128 + p]
    t = b.reshape(4, 4, 128)  # [g, j, p]
    out = np.repeat(t.transpose(2, 0, 1).reshape(128, 16)[:, :, None], BL,
                    axis=2)
    return np.ascontiguousarray(out.reshape(128, 16 * BL)).astype(np.float32)


def prepare_in_maps(emb, node_emb_encoded, W1, b1, W2, b2,
                    Wih0, Whh0, bih0, bhh0, Wih1, Whh1, bih1, bhh1,
                    Wo, bo):
    import ml_dtypes
    wb_parts = []
    for W in (Wih0, Whh0, Wih1, Whh1, Wo):
        Wt = _prep_w(np.asarray(W, np.float32))
        hi = _bf16(Wt)
        lo = _bf16(Wt - hi.astype(np.float32))
        wb_parts += [hi, lo]
    wb = np.ascontiguousarray(np.concatenate(wb_parts, axis=1))

    wf = np.concatenate([
        _prep_w(np.asarray(W1)),
        _prep_w(np.asarray(W2)),
        _prep_bias_cols(np.asarray(b1)),
        _prep_bias_cols(np.asarray(b2)),
        _prep_bias_bc(np.asarray(bih0) + np.asarray(bhh0)),
        _prep_bias_bc(np.asarray(bih1) + np.asarray(bhh1)),
        np.asarray(bo).reshape(128, 1).astype(np.float32),
        np.tile(np.arange(N, dtype=np.float32), (128, 1)),
    ], axis=1)
    wf = np.ascontiguousarray(wf.astype(np.float32))

    in_maps = []
    for c in range(NCORES):
        sl = slice(c * BL, (c + 1) * BL)
        emb_sl = np.asarray(emb[sl], dtype=np.float32)
        node_sl = np.asarray(node_emb_encoded[sl], dtype=np.float32)
        embT = emb_sl.T.reshape(4, 128, BL).transpose(1, 0, 2).reshape(128, -1)
        nodeT = node_sl.transpose(2, 0, 1).reshape(128, -1)
        mi = np.zeros((128, N), np.float32)
        mi[:BL] = -0.5 * (node_sl.astype(np.float32) ** 2).sum(axis=2)
        dat = np.ascontiguousarray(
            np.concatenate([embT, nodeT, mi], axis=1).astype(np.float32))
        in_maps.append({"wb": wb, "wf": wf, "dat": dat})
    return in_maps


def run(inputs, reps=1, debug=False):
    from concourse.bass_utils import run_bass_kernel_spmd
    key = (reps, debug)
    if key not in _CACHE:
        _CACHE[key] = _build(reps=reps, debug=debug)
    nc = _CACHE[key]
    in_maps = prepare_in_maps(
        inputs["emb"], inputs["node_emb_encoded"], inputs["W1"], inputs["b1"],
        inputs["W2"], inputs["b2"], inputs["Wih0"], inputs["Whh0"],
        inputs["bih0"], inputs["bhh0"], inputs["Wih1"], inputs["Whh1"],
        inputs["bih1"], inputs["bhh1"], inputs["Wo"], inputs["bo"])
    res = run_bass_kernel_spmd(nc, in_maps, list(range(NCORES)))
    return res.results


def kernel(**inputs) -> np.ndarray:
    results = run(inputs, reps=1, debug=False)
    out = np.concatenate([r["outT"] for r in results], axis=0)
    return out.astype(np.float32)
